# revision 2
# baseline (speedup 1.0000x reference)
"""Trainium2 Bass kernel v2 for nn_EntropywithDis (geo contrastive loss).

Same math as the baseline kernel, rescheduled for engine density:
  - ONE batched indirect gather for the 160-point pools ([128,80] offsets)
    and ONE for the selected negatives ([128,16]) instead of 96 calls
    (994ns SWDGE fixed cost per call).
  - inputs packed host-side into 3 DMA loads (spack / wpackA / wpackB).
  - imgs pre-transposed on host; img_embT computed directly (no PE
    transposes); weights and big-matmul activations in bf16.
  - ang matmuls in f32r (full rate, exact mantissa) instead of fp32.
  - encoder fused per chunk (ang -> trig -> W1 -> W2 -> norms -> logits),
    chunks sized [64gps, 256, 512, 512, 512, 256].
  - exp deferred to a single post-chunk phase (one ACT table switch);
    logits staged in SBUF as bf16; rank compare/reduce split DVE/Pool.

Outputs per core: se_part [1,B] (sum of exp over its 2112 columns) and
diag_part [BC,1]; host reduces and takes -mean(diag - log(se)).
"""

import math

import numpy as np

import concourse.bass as bass
import concourse.mybir as mybir
import concourse.tile as tile
from concourse import bacc
from concourse.bass import IndirectOffsetOnAxis
from concourse.bass_utils import run_bass_kernel_spmd
from concourse.masks import make_identity

# ---- problem constants (hardcoded per contract) ----
B, Q, NG = 512, 16384, 1_000_000
D_IMG, E, F_DIM, H_DIM = 2048, 512, 256, 1024
PER = 32          # negatives per sample
POOL = 160        # candidate pool per sample
NEAR_CNT = 48     # pool size - num_far_total
N_CORES = 8
BC = B // N_CORES            # 64 batch rows per core
RC = BC + BC * PER           # 2112 encoder columns per core
HP = POOL // 2               # 80 pool points per partition (split layout)
HS = PER // 2                # 16 slots per partition half
DEG = float(np.float32(math.pi / 180.0))
NOISE_STD = float(np.float32(2500.0 / 111320.0))
TWO_PI = float(np.float32(2.0 * math.pi))
PI = float(np.float32(math.pi))
HALF_PI = float(np.float32(math.pi / 2.0))

F32 = mybir.dt.float32
F32R = mybir.dt.float32r
BF16 = mybir.dt.bfloat16
I32 = mybir.dt.int32
AF = mybir.ActivationFunctionType
ALU = mybir.AluOpType
AX = mybir.AxisListType

# encoder chunks over this core's 2112 columns: (c0, c1)
CHUNKS = [(0, 64), (64, 320), (320, 832), (832, 1344), (1344, 1856),
          (1856, 2112)]
N_RT = 17  # total 128-col row-tiles over 2112 columns

# ---- spack column map (one [128, SPK] f32 DMA for all small inputs) ----
_SP = {}
_off = 0
for _name, _w in [
    ("pool_off", HP),        # i32 bits; rows 0:64 = idx[:, :80], 64:128 = idx[:, 80:]
    ("pool_f_full", POOL),   # pool idx as f32, full row per partition
    ("rank_fix_full", POOL),
    ("gps_dup", 2),          # [128, 2] gps rows tiled x2
    ("slot_dup", HS),        # [128, 16] (q,b): q=0 near slots, q=1 far slots
    ("noise_sk", 2 * PER),   # rows 0:64 only
    ("gpst_loc", BC),        # rows 0:2 only ([2, 64] gps transposed)
    ("freqs", F_DIM),        # rows 0:2 only
    ("b1r", H_DIM // 128),
    ("b2r", E // 128),
    ("lgs", 1),              # row 0 only
    ("diagmask", B),         # rows 0:64
]:
    _SP[_name] = (_off, _off + _w)
    _off += _w
SPK = _off

WPA = 2 * (D_IMG // 128) * E   # imgsT + w_img, bf16 [128, WPA]
WPB = (2 * F_DIM) // 128 * H_DIM + (H_DIM // 128) * E  # w1 + w2


def _newton_rsqrt(nc, pool, src_ap, out_ap, shape):
    """out = 1/sqrt(src), elementwise, DVE only (quake seed + 3 Newtons)."""
    p, f = shape
    ivals = pool.tile([p, f], I32, tag="nt_i")
    y = pool.tile([p, f], F32, tag="nt_y")
    qh = pool.tile([p, f], F32, tag="nt_qh")
    t = pool.tile([p, f], F32, tag="nt_t")
    t2 = pool.tile([p, f], F32, tag="nt_t2")
    nc.vector.tensor_scalar(
        ivals[:], src_ap.bitcast(I32), 1, None, op0=ALU.arith_shift_right
    )
    nc.vector.tensor_scalar(
        ivals[:], ivals[:], -1, 0x5F3759DF, op0=ALU.mult, op1=ALU.add
    )
    nc.vector.tensor_copy(y[:], ivals[:].bitcast(F32))
    nc.vector.tensor_scalar_mul(qh[:], src_ap, 0.5)
    for _ in range(3):
        nc.vector.tensor_mul(t[:], y[:], y[:])
        nc.vector.tensor_mul(t2[:], t[:], qh[:])
        nc.vector.tensor_scalar(
            t[:], t2[:], -1.0, 1.5, op0=ALU.mult, op1=ALU.add
        )
        nc.vector.tensor_mul(y[:], y[:], t[:])
    nc.vector.tensor_copy(out_ap, y[:])


def build_program():
    nc = bacc.Bacc(
        "TRN2", target_bir_lowering=False, debug=False, num_devices=N_CORES
    )

    gallery_d = nc.dram_tensor("gallery", [NG, 2], F32, kind="ExternalInput").ap()
    spack_d = nc.dram_tensor("spack", [128, SPK], F32, kind="ExternalInput").ap()
    wpa_d = nc.dram_tensor("wpa", [128, WPA], BF16, kind="ExternalInput").ap()
    wpb_d = nc.dram_tensor("wpb", [128, WPB], BF16, kind="ExternalInput").ap()

    se_part_d = nc.dram_tensor("se_part", [1, B], F32, kind="ExternalOutput").ap()
    diag_part_d = nc.dram_tensor(
        "diag_part", [BC, 1], F32, kind="ExternalOutput"
    ).ap()

    def sp(name, tile_):
        a, b_ = _SP[name]
        return tile_[:, a:b_]

    with tile.TileContext(nc) as tc:
        with (
            tc.tile_pool(name="consts", bufs=1) as cpool,
            tc.tile_pool(name="psA", bufs=1, space="PSUM") as psA,
            tc.tile_pool(name="psMM", bufs=3, space="PSUM") as psMM,
            tc.tile_pool(name="psSum", bufs=1, space="PSUM") as psSum,
            tc.tile_pool(name="psNq", bufs=1, space="PSUM") as psNq,
            tc.tile_pool(name="psT", bufs=1, space="PSUM") as psT,
        ):
            # ---------- input DMAs (issue order = priority) ----------
            spack = cpool.tile([128, SPK], F32)
            nc.sync.dma_start(spack[:], spack_d)
            wpa = cpool.tile([128, WPA], BF16)
            nc.sync.dma_start(wpa[:], wpa_d)
            wpb = cpool.tile([128, WPB], BF16)
            nc.sync.dma_start(wpb[:], wpb_d)

            KT_I = D_IMG // 128  # 16
            imgsT = wpa[:, : KT_I * E].rearrange("p (t e) -> p t e", t=KT_I)
            w_img = wpa[:, KT_I * E :].rearrange("p (t e) -> p t e", t=KT_I)
            w1 = wpb[:, : 4 * H_DIM].rearrange("p (t h) -> p t h", t=4)
            w2 = wpb[:, 4 * H_DIM :].rearrange("p (t e) -> p t e", t=8)

            # ---------- constants ----------
            _consts = {}

            def constp(val, p=128):
                if val not in _consts:
                    t = cpool.tile([128, 1], F32, tag=f"const{len(_consts)}")
                    nc.gpsimd.memset(t[:], float(val))
                    _consts[val] = t
                return _consts[val][:p, :]

            id128 = cpool.tile([128, 128], F32)
            make_identity(nc, id128[:])
            id64 = cpool.tile([64, 64], F32)
            make_identity(nc, id64[:])
            id1 = cpool.tile([1, 1], F32)
            nc.gpsimd.memset(id1[:], 1.0)
            ones32 = cpool.tile([128, 1], F32)
            nc.gpsimd.memset(ones32[:], 1.0)
            ones = cpool.tile([128, 1], F32R)
            nc.vector.tensor_copy(ones[:], ones32[:])
            ones_row32 = cpool.tile([1, 128], F32)
            nc.gpsimd.memset(ones_row32[:], 1.0)
            ones_row = cpool.tile([1, 128], F32R)
            nc.vector.tensor_copy(ones_row[:], ones_row32[:])
            lgs128 = cpool.tile([128, 1], F32)
            nc.gpsimd.partition_broadcast(lgs128[:], sp("lgs", spack)[0:1, :])
            freqs_r = cpool.tile([2, F_DIM], F32R)
            nc.vector.tensor_copy(freqs_r[:], sp("freqs", spack)[0:2, :])

            coordsT = cpool.tile([2, RC], F32R)
            img_embT = cpool.tile([128, E // 128, B], BF16)
            logits_sb = cpool.tile([128, N_RT, B], BF16)
            s_rt = cpool.tile([128, 20], F32)
            nq_rt = cpool.tile([128, 20], F32)
            diag_raw = cpool.tile([BC, 1], F32)
            diag_sb = cpool.tile([BC, 1], F32)
            se_sb = cpool.tile([1, B], F32)

            # gps coords into coordsT cols 0:64 (DVE, needs only spack)
            nc.vector.tensor_copy(
                coordsT[:, 0:BC], sp("gpst_loc", spack)[0:2, :]
            )

            # =====================================================
            # Mining: batched gather, haversine, rank, select
            # =====================================================
            mp_cm = tc.tile_pool(name="mine", bufs=1)
            mp = mp_cm.__enter__()

            pg = mp.tile([128, HP, 2], F32)
            nc.gpsimd.indirect_dma_start(
                out=pg[:, :, :],
                out_offset=None,
                in_=gallery_d,
                in_offset=IndirectOffsetOnAxis(
                    ap=sp("pool_off", spack).bitcast(I32), axis=0
                ),
            )

            # haversine argument h (monotone in distance), split layout
            gps_dup = sp("gps_dup", spack)
            lat1d = mp.tile([128, 1], F32)
            nc.vector.tensor_scalar_mul(lat1d[:], gps_dup[:, 0:1], DEG)
            lon1d = mp.tile([128, 1], F32)
            nc.vector.tensor_scalar_mul(lon1d[:], gps_dup[:, 1:2], DEG)
            blat = mp.tile([128, 1], F32)
            nc.vector.tensor_scalar_mul(blat[:], lat1d[:], -0.5)
            blon = mp.tile([128, 1], F32)
            nc.vector.tensor_scalar_mul(blon[:], lon1d[:], -0.5)
            clat1 = mp.tile([128, 1], F32)
            nc.scalar.activation(clat1[:], lat1d[:], AF.Sin, bias=constp(HALF_PI))

            lat2 = pg[:, :, 0:1].rearrange("p i one -> p (i one)")
            lon2 = pg[:, :, 1:2].rearrange("p i one -> p (i one)")
            sdlat = mp.tile([128, HP], F32)
            nc.scalar.activation(
                sdlat[:], lat2, AF.Sin, bias=blat[:], scale=constp(DEG / 2)
            )
            s2dlat = mp.tile([128, HP], F32)
            nc.scalar.activation(s2dlat[:], sdlat[:], AF.Square)
            clat2 = mp.tile([128, HP], F32)
            nc.scalar.activation(
                clat2[:], lat2, AF.Sin, bias=constp(HALF_PI), scale=constp(DEG)
            )
            cc12 = mp.tile([128, HP], F32)
            nc.vector.tensor_scalar_mul(cc12[:], clat2[:], clat1[:])
            sdlon = mp.tile([128, HP], F32)
            nc.scalar.activation(
                sdlon[:], lon2, AF.Sin, bias=blon[:], scale=constp(DEG / 2)
            )
            s2dlon = mp.tile([128, HP], F32)
            nc.scalar.activation(s2dlon[:], sdlon[:], AF.Square)
            h2b = mp.tile([128, HP], F32)
            nc.vector.tensor_mul(h2b[:], cc12[:], s2dlon[:])
            nc.vector.tensor_add(h2b[:], h2b[:], s2dlat[:])

            # full per-row copy: h2[(q,b), j] = h[b, j] for all 160 j
            h2 = mp.tile([128, POOL], F32)
            nc.sync.dma_start(h2[0:BC, 0:HP], h2b[0:BC, :])
            nc.sync.dma_start(h2[BC:128, 0:HP], h2b[0:BC, :])
            nc.sync.dma_start(h2[0:BC, HP:POOL], h2b[BC:128, :])
            nc.sync.dma_start(h2[BC:128, HP:POOL], h2b[BC:128, :])

            # rank[i] = #{j : h_j < h_i}; compares on Pool, reduces on DVE
            # (gpsimd tensor_reduce cannot reduce the free axis)
            HB = HP // 4  # 20 i-columns per pass
            rank2 = mp.tile([128, HP], F32)
            for qh in range(4):
                cmp3 = mp.tile([128, HB, POOL], BF16, tag=f"cmp{qh % 2}")
                in0 = h2[:].unsqueeze(1).to_broadcast([128, HB, POOL])
                in1 = (
                    h2b[:, qh * HB : (qh + 1) * HB]
                    .unsqueeze(2)
                    .to_broadcast([128, HB, POOL])
                )
                nc.vector.tensor_tensor(
                    out=cmp3[:], in0=in0, in1=in1, op=ALU.is_lt
                )
                nc.vector.tensor_reduce(
                    out=rank2[:, qh * HB : (qh + 1) * HB],
                    in_=cmp3[:], axis=AX.X, op=ALU.add,
                )

            # full rank row per partition + stable-duplicate fix
            rank_full = mp.tile([128, POOL], F32)
            nc.sync.dma_start(rank_full[0:BC, 0:HP], rank2[0:BC, :])
            nc.sync.dma_start(rank_full[BC:128, 0:HP], rank2[0:BC, :])
            nc.sync.dma_start(rank_full[0:BC, HP:POOL], rank2[BC:128, :])
            nc.sync.dma_start(rank_full[BC:128, HP:POOL], rank2[BC:128, :])
            nc.vector.tensor_add(
                rank_full[:], rank_full[:], sp("rank_fix_full", spack)
            )

            # slot match -> gallery row index of each selected negative
            eqm = mp.tile([128, HS, POOL], BF16)
            nc.vector.tensor_tensor(
                out=eqm[:],
                in0=rank_full[:].unsqueeze(1).to_broadcast([128, HS, POOL]),
                in1=sp("slot_dup", spack)
                .unsqueeze(2)
                .to_broadcast([128, HS, POOL]),
                op=ALU.is_equal,
            )
            prodm = mp.tile([128, HS, POOL], F32)
            nc.vector.scalar_tensor_tensor(
                out=prodm[:], in0=eqm[:], scalar=1.0,
                in1=sp("pool_f_full", spack)
                .unsqueeze(1)
                .to_broadcast([128, HS, POOL]),
                op0=ALU.mult, op1=ALU.mult,
            )
            offs2f = mp.tile([128, HS], F32)
            nc.vector.tensor_reduce(
                out=offs2f[:], in_=prodm[:], axis=AX.X, op=ALU.add
            )
            off_n = mp.tile([128, HS], I32)
            nc.vector.tensor_copy(off_n[:], offs2f[:])

            ng = mp.tile([128, HS, 2], F32)
            nc.gpsimd.indirect_dma_start(
                out=ng[:, :, :],
                out_offset=None,
                in_=gallery_d,
                in_offset=IndirectOffsetOnAxis(ap=off_n[:, :], axis=0),
            )

            negs0 = mp.tile([BC, PER, 2], F32)
            nc.sync.dma_start(negs0[:, 0:HS, :], ng[0:BC, :, :])
            nc.sync.dma_start(negs0[:, HS:PER, :], ng[BC:128, :, :])
            negs1 = mp.tile([BC, 2 * PER], F32)
            nc.vector.tensor_add(
                negs1[:],
                negs0[:].rearrange("b s c -> b (s c)"),
                sp("noise_sk", spack)[0:BC, :],
            )
            negs2 = mp.tile([BC, 2 * PER], F32)
            nc.vector.tensor_copy(
                negs2[:].rearrange("b (two s) -> b two s", two=2),
                negs1[:].rearrange("b (s two) -> b two s", two=2),
            )
            # NOTE: negsT transpose (a PE op) is emitted AFTER the image
            # branch so it does not block the PE queue head during mining.

            # =====================================================
            # Image branch: img_embT = l2norm(imgs@W_img).T  (bf16)
            # =====================================================
            ip_cm = tc.tile_pool(name="imgp", bufs=1)
            ip = ip_cm.__enter__()
            pnq_i = psNq.tile([1, B], F32, tag="nq")
            emb_raw = ip.tile([128, E // 128, B], BF16)
            for et in range(E // 128):
                pim = psMM.tile([128, B], F32, tag="mm")
                for kt in range(KT_I):
                    nc.tensor.matmul(
                        pim[:],
                        lhsT=w_img[:, kt, et * 128 : (et + 1) * 128],
                        rhs=imgsT[:, kt, :],
                        start=(kt == 0),
                        stop=(kt == KT_I - 1),
                    )
                sq = ip.tile([128, B], F32R, tag="isq")
                nc.scalar.activation(sq[:], pim[:], AF.Square)
                nc.tensor.matmul(
                    pnq_i[:], lhsT=ones[:], rhs=sq[:],
                    start=(et == 0), stop=(et == E // 128 - 1),
                )
                nc.scalar.copy(emb_raw[:, et, :], pim[:])
            nqi_sb = ip.tile([1, B], F32)
            nc.vector.tensor_copy(nqi_sb[:], pnq_i[:])
            # [1,512] -> [128,4], newton, back to [1,512], outer-bcast
            nqi_rt = ip.tile([128, B // 128], F32)
            for t in range(B // 128):
                pst = psT.tile([128, 128], F32, tag="tps")
                nc.tensor.transpose(
                    pst[:, 0:1], nqi_sb[0:1, t * 128 : (t + 1) * 128], id1[:]
                )
                nc.vector.tensor_copy(nqi_rt[:, t : t + 1], pst[:, 0:1])
            rni_rt = ip.tile([128, B // 128], F32)
            _newton_rsqrt(nc, ip, nqi_rt[:], rni_rt[:], (128, B // 128))
            rni_row = ip.tile([1, B], F32R)
            for t in range(B // 128):
                pst = psT.tile([128, 128], F32, tag="tps")
                nc.tensor.transpose(
                    pst[0:1, :], rni_rt[:, t : t + 1], id128[:]
                )
                nc.vector.tensor_copy(
                    rni_row[:, t * 128 : (t + 1) * 128], pst[0:1, :]
                )
            rn_bc = psMM.tile([128, B], F32, tag="mm")
            nc.tensor.matmul(
                rn_bc[:], lhsT=ones_row[:], rhs=rni_row[:],
                start=True, stop=True,
            )
            for et in range(E // 128):
                nc.vector.tensor_mul(
                    img_embT[:, et, :], emb_raw[:, et, :], rn_bc[:]
                )

            # negatives -> coordsT (PE transpose placed after image matmuls)
            ps_n = psT.tile([64, 64], F32, tag="tps")
            nc.tensor.transpose(ps_n[:], negs2[:], id64[:])
            negsT = ip.tile([64, 64], F32R)
            nc.vector.tensor_copy(negsT[:], ps_n[:])
            nc.sync.dma_start(coordsT[0:1, BC:RC], negsT[0:32, :])
            nc.sync.dma_start(coordsT[1:2, BC:RC], negsT[32:64, :])

            ip_cm.__exit__(None, None, None)
            mp_cm.__exit__(None, None, None)

            # =====================================================
            # Encoder + logits, fused per chunk
            # =====================================================
            ep_cm = tc.tile_pool(name="enc", bufs=2)
            ep = ep_cm.__enter__()
            eps_cm = tc.tile_pool(name="encs", bufs=2)
            eps = eps_cm.__enter__()

            rt_global = 0
            for ci, (c0, c1) in enumerate(CHUNKS):
                cw = c1 - c0
                # --- ang (f32r, exact) ---
                pang = psA.tile([128, 2, 512], F32, tag="ang")
                for m in range(2):
                    nc.tensor.matmul(
                        pang[:, m, :cw],
                        lhsT=freqs_r[:, m * 128 : (m + 1) * 128],
                        rhs=coordsT[:, c0:c1],
                        start=True, stop=True,
                    )
                # --- trig prep (DVE) + sin/cos (scalar) ---
                ki = eps.tile([128, 2, 512], I32, tag="ki")
                nc.vector.tensor_scalar(
                    ki[:, :, :cw], pang[:, :, :cw], 1.0 / TWO_PI, None,
                    op0=ALU.mult,
                )
                kf = eps.tile([128, 2, 512], F32, tag="kf")
                nc.vector.tensor_copy(kf[:, :, :cw], ki[:, :, :cw])
                mscr = eps.tile([128, 2, 512], F32, tag="mscr")
                nc.vector.scalar_tensor_tensor(
                    out=mscr[:, :, :cw], in0=kf[:, :, :cw], scalar=-TWO_PI,
                    in1=pang[:, :, :cw], op0=ALU.mult, op1=ALU.add,
                )
                wrap = eps.tile([128, 2, 512], F32, tag="wrap")
                nc.vector.add_range_wrap(
                    wrap[:, :, :cw], mscr[:, :, :cw], 0.0, PI, TWO_PI
                )
                wrap2 = eps.tile([128, 2, 512], F32, tag="wrap2")
                nc.vector.add_range_wrap(
                    wrap2[:, :, :cw], mscr[:, :, :cw], HALF_PI, PI, TWO_PI
                )
                ffc = ep.tile([128, 4, 512], BF16, tag="ffc")
                nc.scalar.activation(
                    ffc[:, 0:2, :cw], wrap[:, :, :cw], AF.Sin
                )
                nc.scalar.activation(
                    ffc[:, 2:4, :cw], wrap2[:, :, :cw], AF.Sin
                )
                # --- h = relu(ff @ W1 + b1) ---
                hc = ep.tile([128, H_DIM // 128, 512], BF16, tag="hc")
                for mh in range(H_DIM // 128):
                    ph = psMM.tile([128, 512], F32, tag="mm")
                    for kt in range(4):
                        nc.tensor.matmul(
                            ph[:, :cw],
                            lhsT=w1[:, kt, mh * 128 : (mh + 1) * 128],
                            rhs=ffc[:, kt, :cw],
                            start=(kt == 0), stop=(kt == 3),
                        )
                    nc.scalar.activation(
                        hc[:, mh, :cw], ph[:, :cw], AF.Relu,
                        bias=sp("b1r", spack)[:, mh : mh + 1],
                    )
                # --- g = h @ W2 + b2 (bf16) + col norms ---
                gc = ep.tile([128, E // 128, 512], BF16, tag="gc")
                pnq = psNq.tile([1, 512], F32, tag="nq")
                for me in range(E // 128):
                    pg2 = psMM.tile([128, 512], F32, tag="mm")
                    for kt in range(H_DIM // 128):
                        nc.tensor.matmul(
                            pg2[:, :cw],
                            lhsT=w2[:, kt, me * 128 : (me + 1) * 128],
                            rhs=hc[:, kt, :cw],
                            start=(kt == 0), stop=(kt == H_DIM // 128 - 1),
                        )
                    nc.scalar.activation(
                        gc[:, me, :cw], pg2[:, :cw], AF.Identity,
                        bias=sp("b2r", spack)[:, me : me + 1],
                    )
                    sqc = eps.tile([128, 512], F32R, tag="sqc")
                    nc.vector.tensor_mul(
                        sqc[:, :cw], gc[:, me, :cw], gc[:, me, :cw]
                    )
                    nc.tensor.matmul(
                        pnq[:, :cw], lhsT=ones[:], rhs=sqc[:, :cw],
                        start=(me == 0), stop=(me == E // 128 - 1),
                    )
                nq_sb = eps.tile([1, 512], F32, tag="nqsb")
                nc.vector.tensor_copy(nq_sb[:, :cw], pnq[:, :cw])
                n_rt = (cw + 127) // 128
                for t in range(n_rt):
                    rw = min(128, cw - t * 128)
                    pst = psT.tile([128, 128], F32, tag="tps")
                    nc.tensor.transpose(
                        pst[:rw, 0:1],
                        nq_sb[0:1, t * 128 : t * 128 + rw],
                        id1[:],
                    )
                    nc.vector.tensor_copy(
                        nq_rt[:rw, rt_global + t : rt_global + t + 1],
                        pst[:rw, 0:1],
                    )
                # fixed 4-wide newton (junk in pad columns is never read)
                rs_c = eps.tile([128, 4], F32, tag="rs_c")
                _newton_rsqrt(
                    nc, eps,
                    nq_rt[:, rt_global : rt_global + 4],
                    rs_c[:, :4], (128, 4),
                )
                nc.vector.tensor_scalar_mul(
                    s_rt[:, rt_global : rt_global + n_rt],
                    rs_c[:, :n_rt], lgs128[:],
                )
                # --- logits (transposed): pl[col, b] ---
                for t in range(n_rt):
                    rw = min(128, cw - t * 128)
                    pl = psMM.tile([128, B], F32, tag="mm")
                    for et in range(E // 128):
                        nc.tensor.matmul(
                            pl[:rw, :],
                            lhsT=gc[:, et, t * 128 : t * 128 + rw],
                            rhs=img_embT[:, et, :],
                            start=(et == 0), stop=(et == E // 128 - 1),
                        )
                    if ci == 0 and t == 0:
                        dm = eps.tile([BC, B], F32, tag="dm")
                        nc.vector.tensor_mul(
                            dm[:], pl[0:BC, :], sp("diagmask", spack)[0:BC, :]
                        )
                        nc.vector.tensor_reduce(
                            out=diag_raw[:], in_=dm[:], axis=AX.X, op=ALU.add
                        )
                    nc.scalar.copy(
                        logits_sb[:rw, rt_global + t, :], pl[:rw, :]
                    )
                rt_global += n_rt

            # diag scale (needs s_rt of chunk 0)
            nc.vector.tensor_scalar_mul(
                diag_sb[:], diag_raw[:], s_rt[0:BC, 0:1]
            )

            # =====================================================
            # Exp + partial sum-of-exp (one ACT table switch)
            # =====================================================
            se_ps = psSum.tile([1, B], F32)
            rws = []
            for (c0, c1) in CHUNKS:
                cw = c1 - c0
                for t in range((cw + 127) // 128):
                    rws.append(min(128, cw - t * 128))
            for rt in range(N_RT):
                rw = rws[rt]
                expt = ep.tile([128, B], F32R, tag="expt")
                nc.scalar.activation(
                    expt[:rw, :], logits_sb[:rw, rt, :], AF.Exp,
                    scale=s_rt[:rw, rt : rt + 1],
                )
                nc.tensor.matmul(
                    se_ps[:], lhsT=ones[:rw, :], rhs=expt[:rw, :],
                    start=(rt == 0), stop=(rt == N_RT - 1),
                )
            nc.vector.tensor_copy(se_sb[:], se_ps[:])

            nc.sync.dma_start(se_part_d, se_sb[:])
            nc.sync.dma_start(diag_part_d, diag_sb[:])

            eps_cm.__exit__(None, None, None)
            ep_cm.__exit__(None, None, None)

    nc.compile()
    return nc


_PROGRAM = None


def _get_program():
    global _PROGRAM
    if _PROGRAM is None:
        _PROGRAM = build_program()
    return _PROGRAM


def make_in_maps(inputs):
    import ml_dtypes

    bf16 = ml_dtypes.bfloat16
    imgs = np.asarray(inputs["imgs"], np.float32)
    gps = np.asarray(inputs["gps"], np.float32)
    gallery = np.ascontiguousarray(np.asarray(inputs["gps_gallery"], np.float32))
    w_img = np.asarray(inputs["W_img"], np.float32)
    freqs = np.asarray(inputs["freqs"], np.float32)
    w1 = np.asarray(inputs["W1"], np.float32)
    b1 = np.asarray(inputs["b1"], np.float32)
    w2 = np.asarray(inputs["W2"], np.float32)
    b2 = np.asarray(inputs["b2"], np.float32)
    lgs = float(np.asarray(inputs["logit_scale"], np.float32))
    pool_idx = np.asarray(inputs["pool_idx"], np.int32)
    far_sel = np.asarray(inputs["far_sel"], np.int32)
    perm = np.asarray(inputs["perm"], np.int64)

    # deterministic noise constant (jax PRNG, key=1), permuted to neg order.
    import jax
    import jax.numpy as jnp

    try:
        cpu_dev = jax.local_devices(backend="cpu")[0]
        ctx = jax.default_device(cpu_dev)
    except RuntimeError:
        import contextlib

        ctx = contextlib.nullcontext()
    with ctx:
        noise = np.asarray(
            jax.random.normal(jax.random.key(1), (Q, 2), jnp.float32)
        ) * np.float32(NOISE_STD)
    assert np.array_equal(np.sort(perm), np.arange(Q)), "perm not a permutation"
    noise_p = noise[perm]

    # stable-rank fix for duplicate pool indices within a row
    eq = pool_idx[:, :, None] == pool_idx[:, None, :]
    tril = np.tril(np.ones((POOL, POOL), bool), -1)[None]
    rank_fix = (eq & tril).sum(axis=2).astype(np.float32)

    # weight packs (bf16)
    KT_I = D_IMG // 128
    imgsT_p = np.ascontiguousarray(imgs.T).reshape(KT_I, 128, E)
    imgsT_p = imgsT_p.transpose(1, 0, 2).reshape(128, KT_I * E)
    w_img_p = w_img.reshape(KT_I, 128, E).transpose(1, 0, 2).reshape(128, -1)
    w1_p = w1.reshape(4, 128, H_DIM).transpose(1, 0, 2).reshape(128, -1)
    w2_p = w2.reshape(8, 128, E).transpose(1, 0, 2).reshape(128, -1)
    wpa = np.ascontiguousarray(
        np.concatenate([imgsT_p, w_img_p], axis=1)
    ).astype(bf16)
    wpb = np.ascontiguousarray(
        np.concatenate([w1_p, w2_p], axis=1)
    ).astype(bf16)

    b1r = np.ascontiguousarray(b1.reshape(H_DIM // 128, 128).T)
    b2r = np.ascontiguousarray(b2.reshape(E // 128, 128).T)

    in_maps = []
    for c in range(N_CORES):
        rows = slice(c * BC, (c + 1) * BC)
        spk = np.zeros((128, SPK), np.float32)

        def put(name, arr, p0=0):
            a, b_ = _SP[name]
            arr = np.asarray(arr)
            assert arr.shape[1] == b_ - a, (name, arr.shape)
            spk[p0 : p0 + arr.shape[0], a:b_] = arr

        po = np.concatenate(
            [pool_idx[rows, :HP], pool_idx[rows, HP:]], axis=0
        ).astype(np.int32)
        put("pool_off", po.view(np.float32))
        put("pool_f_full", np.tile(pool_idx[rows].astype(np.float32), (2, 1)))
        put("rank_fix_full", np.tile(rank_fix[rows], (2, 1)))
        put("gps_dup", np.tile(gps[rows], (2, 1)))
        slot_dup = np.concatenate(
            [
                np.tile(np.arange(HS, dtype=np.float32), (BC, 1)),
                (NEAR_CNT + far_sel[rows]).astype(np.float32),
            ],
            axis=0,
        )
        put("slot_dup", slot_dup)
        ns = noise_p[c * BC * PER : (c + 1) * BC * PER].reshape(BC, 2 * PER)
        put("noise_sk", ns)
        put("gpst_loc", np.ascontiguousarray(gps[rows].T))
        put("freqs", freqs)
        put("b1r", b1r)
        put("b2r", b2r)
        put("lgs", np.array([[lgs]], np.float32))
        dm = np.zeros((BC, B), np.float32)
        dm[np.arange(BC), c * BC + np.arange(BC)] = 1.0
        put("diagmask", dm)

        in_maps.append(
            {"gallery": gallery, "spack": spk, "wpa": wpa, "wpb": wpb}
        )
    return in_maps


def kernel(**inputs):
    nc = _get_program()
    in_maps = make_in_maps(inputs)
    res = run_bass_kernel_spmd(nc, in_maps, list(range(N_CORES)))
    se = np.zeros((1, B), np.float64)
    dg = np.zeros(B, np.float64)
    for c in range(N_CORES):
        se += res.results[c]["se_part"]
        dg[c * BC : (c + 1) * BC] = res.results[c]["diag_part"][:, 0]
    loss = -np.mean(dg - np.log(se.reshape(-1)))
    return np.float32(loss)
